# revision 11
# baseline (speedup 1.0000x reference)
"""Multi-head self-attention Trainium2 kernel (Bass/Tile), batch-sharded SPMD.

Problem: seq [2048, 8, 512] fp32, fused QKV (W_qkv [1536,512], b_qkv [1536]),
H=8 heads of HD=64, full softmax attention, out proj (W_out [512,512], b_out).

Sharding: batch (bs=8) across 8 NeuronCores, one batch element per core.
No collectives; host scatters seq[:, b, :] (pre-transposed to [e, n]) and
gathers y -> [n, bs, e]. Weights are pre-transposed on host too, so no
on-chip transposes at all.

Per-core dataflow (n=2048, E=512):
  xT    [e, n]   <- DMA fp32, cast bf16 on DVE
  qkT   [f, n]   <- WqkvT.T @ xT  (f in [0,1024): q|k features; head-pairs
                    per 128-row tile: rows 0:64 head 2p, 64:128 head 2p+1)
  v     [n, f]   <- xT.T @ WvT (+bias via ones-lhsT matmul)
  per head pair p, per q-chunk (512 cols), k-blocks in batches of KBATCH:
    scoresT[k,q]: row-tiled PAIR matmuls (K=64 halves run concurrently)
    exp on ScalarE (scale=1/8, no max subtraction: |s| < ~4, exp safe)
    outT[hd,q] += v[k,hd].T @ exp   (col-tiled pair: M=64 at cols 0/64)
    denom      += ones[k,64].T @ exp (PE broadcasts denom over 64 rows)
    outT_norm   = outT * reciprocal(denom)  (both PSUM tiles double-buffered
                  so the slow DVE reciprocal stays off the PE critical path)
  y[n, f] = outT.T @ WoutT + b_out (ones-lhsT matmul)
"""

import numpy as np

import concourse.bass as bass
import concourse.mybir as mybir
import concourse.tile as tile
from concourse import bacc

F32 = mybir.dt.float32
BF16 = mybir.dt.bfloat16

N_SEQ, BS, E, H, HD = 2048, 8, 512, 8, 64
N_CORES = 8
KBATCH = 2  # k-blocks per scores PSUM tile (2+2 banks + 2*out + 2*denom = 8)


def _emit(tc, nc, xT_d, w_qkvT, b_qkv, w_outT, b_out, y, n):
    NB = n // 128   # token blocks
    QC = n // 512   # q chunks
    KB = n // 128   # k blocks
    EC = E // 128   # e chunks

    persist_cm = tc.tile_pool(name="persist", bufs=1)
    persist = persist_cm.__enter__()

    ones_col = persist.tile([128, 64], BF16, tag="ones_col", name="ones_col")
    nc.vector.memset(ones_col, 1.0)
    ones_row = persist.tile([1, 128], BF16, tag="ones_row", name="ones_row")
    nc.vector.memset(ones_row, 1.0)

    # biases: b_qkv[0:1024] per-partition layout [128, fb]; v/out biases as rows
    bqk = persist.tile([128, 8], F32, tag="bqk", name="bqk")
    nc.sync.dma_start(out=bqk, in_=b_qkv[0:1024].rearrange("(a b) -> b a", b=128))
    bv_f = persist.tile([1, 512], F32, tag="bv_f", name="bv_f")
    nc.sync.dma_start(out=bv_f, in_=b_qkv[1024:1536].unsqueeze(0))
    bv = persist.tile([1, 512], BF16, tag="bv", name="bv")
    nc.vector.tensor_copy(bv, bv_f)
    bo_f = persist.tile([1, 512], F32, tag="bo_f", name="bo_f")
    nc.sync.dma_start(out=bo_f, in_=b_out.unsqueeze(0))
    bo = persist.tile([1, 512], BF16, tag="bo", name="bo")
    nc.vector.tensor_copy(bo, bo_f)

    # persistent bf16 operands
    xT = persist.tile([128, EC, n], BF16, tag="xT", name="xT")
    wqkvT = persist.tile([128, EC, 1536], BF16, tag="wqkvT", name="wqkvT")
    woutT = persist.tile([128, EC, 512], BF16, tag="woutT", name="woutT")
    qkT = [persist.tile([128, n], BF16, tag=f"qkT{i}", name=f"qkT{i}") for i in range(8)]
    v_sb = [persist.tile([128, 512], BF16, tag=f"v{i}", name=f"v{i}") for i in range(NB)]
    outT = [persist.tile([128, n], BF16, tag=f"outT{p}", name=f"outT{p}") for p in range(4)]

    # ---------------- phase 0: load (bf16, pre-transposed on host) + QKV ----
    with (
        tc.tile_pool(name="pqkv", bufs=4, space="PSUM") as pqkv_pool,
    ):
        for j in range(EC):
            nc.sync.dma_start(
                out=wqkvT[:, j, :], in_=w_qkvT[j * 128:(j + 1) * 128, :]
            )
            nc.sync.dma_start(out=xT[:, j, :], in_=xT_d[j * 128:(j + 1) * 128, :])
        for j in range(EC):
            nc.sync.dma_start(
                out=woutT[:, j, :], in_=w_outT[j * 128:(j + 1) * 128, :]
            )

        def emit_qk(fb):
            for ncol in range(QC):
                pq = pqkv_pool.tile([128, 512], F32, tag="qk", name="pq")
                for j in range(EC):
                    nc.tensor.matmul(
                        pq,
                        lhsT=wqkvT[:, j, fb * 128:(fb + 1) * 128],
                        rhs=xT[:, j, ncol * 512:(ncol + 1) * 512],
                        start=(j == 0),
                        stop=(j == EC - 1),
                    )
                nc.vector.tensor_scalar_add(
                    qkT[fb][:, ncol * 512:(ncol + 1) * 512], pq, bqk[:, fb:fb + 1]
                )

        def emit_v(nb):
            pv = pqkv_pool.tile([128, 512], F32, tag="v", name="pv")
            for j in range(EC):
                nc.tensor.matmul(
                    pv,
                    lhsT=xT[:, j, nb * 128:(nb + 1) * 128],
                    rhs=wqkvT[:, j, 1024:1536],
                    start=(j == 0),
                    stop=False,
                )
            nc.tensor.matmul(pv, lhsT=ones_row, rhs=bv, start=False, stop=True)
            nc.vector.tensor_copy(v_sb[nb], pv)

        emit_qk(0)
        emit_qk(4)
        for nb in range(NB):
            emit_v(nb)
        for fb in (1, 5, 2, 6, 3, 7):
            emit_qk(fb)

    # ---------------- phase 1: attention ----------------
    # 3-kb cycles over a single 6-bank scores tensor: kb0/kb1 (A,B interleaved)
    # in banks 0-3 -> one 2048-elem exp; kb2 in banks 4-5 -> one 1024-elem exp.
    # The second exp hides the PE time of av(cycle)+scores(next cycle), so
    # ScalarE stays saturated. o/d single-banked; reciprocal_approx_fast makes
    # the qc-boundary normalize cheap enough to hide behind next-qc scores.
    cycles = [tuple(range(s, min(s + 3, KB))) for s in range(0, KB, 3)]
    with (
        tc.tile_pool(name="ps", bufs=1, space="PSUM") as s_pool,
        tc.tile_pool(name="po", bufs=1, space="PSUM") as o_pool,
        tc.tile_pool(name="se", bufs=2) as e_pool,
        tc.tile_pool(name="sr", bufs=2) as r_pool,
    ):
        for p in range(4):
            qa = qkT[p]
            ka = qkT[4 + p]
            for qc in range(QC):
                qs = slice(qc * 512, (qc + 1) * 512)
                po = o_pool.tile([128, 512], F32, tag="o", name="po")
                pd = o_pool.tile([128, 512], F32, tag="d", name="pd")

                def scores(S, slot, kb):
                    ks = slice(kb * 128, (kb + 1) * 128)
                    nc.tensor.matmul(
                        S[:, 2 * slot, :], lhsT=ka[0:64, ks], rhs=qa[0:64, qs],
                        start=True, stop=True,
                    )
                    nc.tensor.matmul(
                        S[:, 2 * slot + 1, :], lhsT=ka[64:128, ks], rhs=qa[64:128, qs],
                        start=True, stop=True,
                    )

                def av(e, slot, kb):
                    first, last = (kb == 0), (kb == KB - 1)
                    eA = e[:, 2 * slot, :]
                    eB = e[:, 2 * slot + 1, :]
                    nc.tensor.matmul(
                        po[0:64, :], lhsT=v_sb[kb][:, p * 128:p * 128 + 64],
                        rhs=eA, start=first, stop=last, skip_group_check=True,
                    )
                    nc.tensor.matmul(
                        po[64:128, :], lhsT=v_sb[kb][:, p * 128 + 64:(p + 1) * 128],
                        rhs=eB, start=first, stop=last, skip_group_check=True,
                    )
                    nc.tensor.matmul(
                        pd[0:64, :], lhsT=ones_col, rhs=eA,
                        start=first, stop=last, skip_group_check=True,
                    )
                    nc.tensor.matmul(
                        pd[64:128, :], lhsT=ones_col, rhs=eB,
                        start=first, stop=last, skip_group_check=True,
                    )

                for cyc in cycles:
                    S = s_pool.tile([128, 6, 512], F32, tag="s", name="S")
                    if len(cyc) == 3:
                        k0, k1, k2 = cyc
                        scores(S, 0, k0)
                        scores(S, 1, k1)
                        e01 = e_pool.tile([128, 4, 512], BF16, tag="e4", name="e01")
                        nc.scalar.activation(
                            e01, S[:, 0:4, :],
                            mybir.ActivationFunctionType.Exp, scale=0.125,
                        )
                        scores(S, 2, k2)
                        e2 = e_pool.tile([128, 2, 512], BF16, tag="e2", name="e2")
                        nc.scalar.activation(
                            e2, S[:, 4:6, :],
                            mybir.ActivationFunctionType.Exp, scale=0.125,
                        )
                        av(e01, 0, k0)
                        av(e01, 1, k1)
                        av(e2, 0, k2)
                    else:
                        (k0,) = cyc
                        scores(S, 0, k0)
                        e2 = e_pool.tile([128, 2, 512], BF16, tag="e2", name="e2")
                        nc.scalar.activation(
                            e2, S[:, 0:2, :],
                            mybir.ActivationFunctionType.Exp, scale=0.125,
                        )
                        av(e2, 0, k0)
                rc = r_pool.tile([128, 512], F32, tag="rc", name="rc")
                nc.vector.reciprocal_approx_fast(rc, pd)
                nc.vector.tensor_mul(outT[p][:, qs], po, rc)

    # ---------------- phase 2: output projection ----------------
    with (
        tc.tile_pool(name="pf", bufs=4, space="PSUM") as f_pool,
        tc.tile_pool(name="sy", bufs=4) as y_pool,
    ):
        for nb in range(NB):
            pf = f_pool.tile([128, 512], F32, tag="f", name="pf")
            for p in range(4):
                nc.tensor.matmul(
                    pf, lhsT=outT[p][:, nb * 128:(nb + 1) * 128],
                    rhs=woutT[:, p, :], start=(p == 0), stop=False,
                )
            nc.tensor.matmul(pf, lhsT=ones_row, rhs=bo, start=False, stop=True)
            ys = y_pool.tile([128, 512], F32, tag="y", name="ys")
            nc.vector.tensor_copy(ys, pf)
            nc.sync.dma_start(out=y[nb * 128:(nb + 1) * 128, :], in_=ys)

    persist_cm.__exit__(None, None, None)


def build(n=N_SEQ):
    nc = bacc.Bacc("TRN2", target_bir_lowering=False, debug=False)
    xT_d = nc.dram_tensor("xT", [E, n], BF16, kind="ExternalInput").ap()
    w_qkvT = nc.dram_tensor("w_qkvT", [E, 3 * E], BF16, kind="ExternalInput").ap()
    b_qkv = nc.dram_tensor("b_qkv", [3 * E], F32, kind="ExternalInput").ap()
    w_outT = nc.dram_tensor("w_outT", [E, E], BF16, kind="ExternalInput").ap()
    b_out = nc.dram_tensor("b_out", [E], F32, kind="ExternalInput").ap()
    y = nc.dram_tensor("y", [n, E], F32, kind="ExternalOutput").ap()
    with tile.TileContext(nc) as tc:
        _emit(tc, nc, xT_d, w_qkvT, b_qkv, w_outT, b_out, y, n)
    nc.compile()
    return nc


_NC_CACHE = {}


def _get_nc(n):
    if n not in _NC_CACHE:
        _NC_CACHE[n] = build(n)
    return _NC_CACHE[n]


def _in_maps(seq, W_qkv, b_qkv, W_out, b_out):
    import ml_dtypes

    bf16 = ml_dtypes.bfloat16
    seq = np.asarray(seq, np.float32)
    wqT = np.ascontiguousarray(np.asarray(W_qkv, np.float32).T.astype(bf16))
    bq = np.ascontiguousarray(np.asarray(b_qkv, np.float32))
    woT = np.ascontiguousarray(np.asarray(W_out, np.float32).T.astype(bf16))
    bo = np.ascontiguousarray(np.asarray(b_out, np.float32))
    return [
        {
            "xT": np.ascontiguousarray(seq[:, b, :].T.astype(bf16)),  # [E, n]
            "w_qkvT": wqT,
            "b_qkv": bq,
            "w_outT": woT,
            "b_out": bo,
        }
        for b in range(seq.shape[1])
    ]


def run(seq, W_qkv, b_qkv, W_out, b_out, trace=False):
    """Returns (out [n, bs, e] fp32, BassKernelResults)."""
    from concourse.bass_utils import run_bass_kernel_spmd

    seq = np.asarray(seq, np.float32)
    n, bs, e = seq.shape
    nc = _get_nc(n)
    res = run_bass_kernel_spmd(
        nc,
        _in_maps(seq, W_qkv, b_qkv, W_out, b_out),
        core_ids=list(range(N_CORES)),
        trace=trace,
    )
    out = np.empty((n, bs, e), np.float32)
    for b in range(bs):
        out[:, b, :] = res.results[b]["y"]
    return out, res


def kernel(seq, W_qkv, b_qkv, W_out, b_out):
    out, _ = run(seq, W_qkv, b_qkv, W_out, b_out)
    return out


# revision 12
# speedup vs baseline: 1.5950x; 1.5950x over previous
"""Multi-head self-attention Trainium2 kernel (Bass/Tile), batch-sharded SPMD.

Problem: seq [2048, 8, 512] fp32, fused QKV (W_qkv [1536,512], b_qkv [1536]),
H=8 heads of HD=64, full softmax attention, out proj (W_out [512,512], b_out).

Sharding: batch (bs=8) across 8 NeuronCores, one batch element per core.
No collectives; host scatters seq[:, b, :] (pre-transposed to [e, n]) and
gathers y -> [n, bs, e]. Weights are pre-transposed on host too, so no
on-chip transposes at all.

Per-core dataflow (n=2048, E=512):
  xT    [e, n]   <- DMA fp32, cast bf16 on DVE
  qkT   [f, n]   <- WqkvT.T @ xT  (f in [0,1024): q|k features; head-pairs
                    per 128-row tile: rows 0:64 head 2p, 64:128 head 2p+1)
  v     [n, f]   <- xT.T @ WvT (+bias via ones-lhsT matmul)
  per head pair p, per q-chunk (512 cols), k-blocks in batches of KBATCH:
    scoresT[k,q]: row-tiled PAIR matmuls (K=64 halves run concurrently)
    exp on ScalarE (scale=1/8, no max subtraction: |s| < ~4, exp safe)
    outT[hd,q] += v[k,hd].T @ exp   (col-tiled pair: M=64 at cols 0/64)
    denom      += ones[k,64].T @ exp (PE broadcasts denom over 64 rows)
    outT_norm   = outT * reciprocal(denom)  (both PSUM tiles double-buffered
                  so the slow DVE reciprocal stays off the PE critical path)
  y[n, f] = outT.T @ WoutT + b_out (ones-lhsT matmul)
"""

import numpy as np

import concourse.bass as bass
import concourse.mybir as mybir
import concourse.tile as tile
from concourse import bacc

F32 = mybir.dt.float32
BF16 = mybir.dt.bfloat16

N_SEQ, BS, E, H, HD = 2048, 8, 512, 8, 64
N_CORES = 8
KBATCH = 2  # k-blocks per scores PSUM tile (2+2 banks + 2*out + 2*denom = 8)


def _emit(tc, nc, xT_d, w_qkvT, b_qkv, w_outT, b_out, y, n):
    NB = n // 128   # token blocks
    QC = n // 512   # q chunks
    KB = n // 128   # k blocks
    EC = E // 128   # e chunks

    persist_cm = tc.tile_pool(name="persist", bufs=1)
    persist = persist_cm.__enter__()

    ones_col = persist.tile([128, 64], BF16, tag="ones_col", name="ones_col")
    nc.vector.memset(ones_col, 1.0)
    ones_row = persist.tile([1, 128], BF16, tag="ones_row", name="ones_row")
    nc.vector.memset(ones_row, 1.0)

    # biases: b_qkv[0:1024] per-partition layout [128, fb]; v/out biases as rows
    bqk = persist.tile([128, 8], F32, tag="bqk", name="bqk")
    nc.sync.dma_start(out=bqk, in_=b_qkv[0:1024].rearrange("(a b) -> b a", b=128))
    bv_f = persist.tile([1, 512], F32, tag="bv_f", name="bv_f")
    nc.sync.dma_start(out=bv_f, in_=b_qkv[1024:1536].unsqueeze(0))
    bv = persist.tile([1, 512], BF16, tag="bv", name="bv")
    nc.vector.tensor_copy(bv, bv_f)
    bo_f = persist.tile([1, 512], F32, tag="bo_f", name="bo_f")
    nc.sync.dma_start(out=bo_f, in_=b_out.unsqueeze(0))
    bo = persist.tile([1, 512], BF16, tag="bo", name="bo")
    nc.vector.tensor_copy(bo, bo_f)

    # persistent bf16 operands
    xT = persist.tile([128, EC, n], BF16, tag="xT", name="xT")
    wqkvT = persist.tile([128, EC, 1536], BF16, tag="wqkvT", name="wqkvT")
    woutT = persist.tile([128, EC, 512], BF16, tag="woutT", name="woutT")
    qkT = [persist.tile([128, n], BF16, tag=f"qkT{i}", name=f"qkT{i}") for i in range(8)]
    v_sb = [persist.tile([128, 512], BF16, tag=f"v{i}", name=f"v{i}") for i in range(NB)]
    outT = [persist.tile([128, n], BF16, tag=f"outT{p}", name=f"outT{p}") for p in range(4)]

    # ---------------- phase 0: load (bf16, pre-transposed on host) + QKV ----
    with (
        tc.tile_pool(name="pqkv", bufs=4, space="PSUM") as pqkv_pool,
    ):
        for j in range(EC):
            nc.sync.dma_start(
                out=wqkvT[:, j, :], in_=w_qkvT[j * 128:(j + 1) * 128, :]
            )
            nc.sync.dma_start(out=xT[:, j, :], in_=xT_d[j * 128:(j + 1) * 128, :])
        for j in range(EC):
            nc.sync.dma_start(
                out=woutT[:, j, :], in_=w_outT[j * 128:(j + 1) * 128, :]
            )

        def emit_qk(fb):
            for ncol in range(QC):
                pq = pqkv_pool.tile([128, 512], F32, tag="qk", name="pq")
                for j in range(EC):
                    nc.tensor.matmul(
                        pq,
                        lhsT=wqkvT[:, j, fb * 128:(fb + 1) * 128],
                        rhs=xT[:, j, ncol * 512:(ncol + 1) * 512],
                        start=(j == 0),
                        stop=(j == EC - 1),
                    )
                nc.vector.tensor_scalar_add(
                    qkT[fb][:, ncol * 512:(ncol + 1) * 512], pq, bqk[:, fb:fb + 1]
                )

        def emit_v(nb):
            pv = pqkv_pool.tile([128, 512], F32, tag="v", name="pv")
            for j in range(EC):
                nc.tensor.matmul(
                    pv,
                    lhsT=xT[:, j, nb * 128:(nb + 1) * 128],
                    rhs=wqkvT[:, j, 1024:1536],
                    start=(j == 0),
                    stop=False,
                )
            nc.tensor.matmul(pv, lhsT=ones_row, rhs=bv, start=False, stop=True)
            nc.vector.tensor_copy(v_sb[nb], pv)

        emit_qk(0)
        emit_qk(4)
        for nb in range(NB):
            emit_v(nb)
        for fb in (1, 5, 2, 6, 3, 7):
            emit_qk(fb)

    # ---------------- phase 1: attention ----------------
    # 3-kb cycles over a single 6-bank scores tensor: kb0/kb1 (A,B interleaved)
    # in banks 0-3 -> one 2048-elem exp; kb2 in banks 4-5 -> one 1024-elem exp.
    # The second exp hides the PE time of av(cycle)+scores(next cycle), so
    # ScalarE stays saturated. o/d single-banked; reciprocal_approx_fast makes
    # the qc-boundary normalize cheap enough to hide behind next-qc scores.
    cycles = [tuple(range(s, min(s + 3, KB))) for s in range(0, KB, 3)]
    with (
        tc.tile_pool(name="ps", bufs=1, space="PSUM") as s_pool,
        tc.tile_pool(name="po", bufs=1, space="PSUM") as o_pool,
        tc.tile_pool(name="se", bufs=2) as e_pool,
        tc.tile_pool(name="sr", bufs=2) as r_pool,
    ):
        for p in range(4):
            qa = qkT[p]
            ka = qkT[4 + p]
            for qc in range(QC):
                qs = slice(qc * 512, (qc + 1) * 512)
                po = o_pool.tile([128, 512], F32, tag="o", name="po")
                pd = o_pool.tile([128, 512], F32, tag="d", name="pd")

                def scores(S, slot, kb):
                    ks = slice(kb * 128, (kb + 1) * 128)
                    nc.tensor.matmul(
                        S[:, 2 * slot, :], lhsT=ka[0:64, ks], rhs=qa[0:64, qs],
                        start=True, stop=True,
                    )
                    nc.tensor.matmul(
                        S[:, 2 * slot + 1, :], lhsT=ka[64:128, ks], rhs=qa[64:128, qs],
                        start=True, stop=True,
                    )

                def av(e, slot, kb):
                    first, last = (kb == 0), (kb == KB - 1)
                    eA = e[:, 2 * slot, :]
                    eB = e[:, 2 * slot + 1, :]
                    nc.tensor.matmul(
                        po[0:64, :], lhsT=v_sb[kb][:, p * 128:p * 128 + 64],
                        rhs=eA, start=first, stop=last, skip_group_check=True,
                    )
                    nc.tensor.matmul(
                        po[64:128, :], lhsT=v_sb[kb][:, p * 128 + 64:(p + 1) * 128],
                        rhs=eB, start=first, stop=last, skip_group_check=True,
                    )
                    nc.tensor.matmul(
                        pd[0:64, :], lhsT=ones_col, rhs=eA,
                        start=first, stop=last, skip_group_check=True,
                    )
                    nc.tensor.matmul(
                        pd[64:128, :], lhsT=ones_col, rhs=eB,
                        start=first, stop=last, skip_group_check=True,
                    )

                pend = []  # av work deferred by one cycle to keep ScalarE fed
                for cyc in cycles:
                    if len(cyc) == 3:
                        k0, k1, k2 = cyc
                        S01 = s_pool.tile([128, 4, 512], F32, tag="s01", name="S01")
                        scores(S01, 0, k0)
                        scores(S01, 1, k1)
                        e1 = e_pool.tile([128, 4, 512], BF16, tag="e4", name="e1")
                        nc.scalar.activation(
                            e1, S01, mybir.ActivationFunctionType.Exp, scale=0.125,
                        )
                        S2 = s_pool.tile([128, 2, 512], F32, tag="s2", name="S2")
                        scores(S2, 0, k2)
                        e2 = e_pool.tile([128, 2, 512], BF16, tag="e2", name="e2")
                        nc.scalar.activation(
                            e2, S2, mybir.ActivationFunctionType.Exp, scale=0.125,
                        )
                        for args in pend:
                            av(*args)
                        pend = [(e1, 0, k0), (e1, 1, k1), (e2, 0, k2)]
                    else:
                        (k0,) = cyc
                        S2 = s_pool.tile([128, 2, 512], F32, tag="s2", name="S2")
                        scores(S2, 0, k0)
                        e2 = e_pool.tile([128, 2, 512], BF16, tag="e2", name="e2")
                        nc.scalar.activation(
                            e2, S2, mybir.ActivationFunctionType.Exp, scale=0.125,
                        )
                        for args in pend:
                            av(*args)
                        pend = [(e2, 0, k0)]
                for args in pend:
                    av(*args)
                rc = r_pool.tile([128, 512], F32, tag="rc", name="rc")
                nc.vector.reciprocal_approx_fast(rc, pd)
                nc.vector.tensor_mul(outT[p][:, qs], po, rc)

    # ---------------- phase 2: output projection ----------------
    with (
        tc.tile_pool(name="pf", bufs=4, space="PSUM") as f_pool,
        tc.tile_pool(name="sy", bufs=4) as y_pool,
    ):
        for nb in range(NB):
            pf = f_pool.tile([128, 512], F32, tag="f", name="pf")
            for p in range(4):
                nc.tensor.matmul(
                    pf, lhsT=outT[p][:, nb * 128:(nb + 1) * 128],
                    rhs=woutT[:, p, :], start=(p == 0), stop=False,
                )
            nc.tensor.matmul(pf, lhsT=ones_row, rhs=bo, start=False, stop=True)
            ys = y_pool.tile([128, 512], F32, tag="y", name="ys")
            nc.vector.tensor_copy(ys, pf)
            nc.sync.dma_start(out=y[nb * 128:(nb + 1) * 128, :], in_=ys)

    persist_cm.__exit__(None, None, None)


def build(n=N_SEQ):
    nc = bacc.Bacc("TRN2", target_bir_lowering=False, debug=False)
    xT_d = nc.dram_tensor("xT", [E, n], BF16, kind="ExternalInput").ap()
    w_qkvT = nc.dram_tensor("w_qkvT", [E, 3 * E], BF16, kind="ExternalInput").ap()
    b_qkv = nc.dram_tensor("b_qkv", [3 * E], F32, kind="ExternalInput").ap()
    w_outT = nc.dram_tensor("w_outT", [E, E], BF16, kind="ExternalInput").ap()
    b_out = nc.dram_tensor("b_out", [E], F32, kind="ExternalInput").ap()
    y = nc.dram_tensor("y", [n, E], F32, kind="ExternalOutput").ap()
    with tile.TileContext(nc) as tc:
        _emit(tc, nc, xT_d, w_qkvT, b_qkv, w_outT, b_out, y, n)
    nc.compile()
    return nc


_NC_CACHE = {}


def _get_nc(n):
    if n not in _NC_CACHE:
        _NC_CACHE[n] = build(n)
    return _NC_CACHE[n]


def _in_maps(seq, W_qkv, b_qkv, W_out, b_out):
    import ml_dtypes

    bf16 = ml_dtypes.bfloat16
    seq = np.asarray(seq, np.float32)
    wqT = np.ascontiguousarray(np.asarray(W_qkv, np.float32).T.astype(bf16))
    bq = np.ascontiguousarray(np.asarray(b_qkv, np.float32))
    woT = np.ascontiguousarray(np.asarray(W_out, np.float32).T.astype(bf16))
    bo = np.ascontiguousarray(np.asarray(b_out, np.float32))
    return [
        {
            "xT": np.ascontiguousarray(seq[:, b, :].T.astype(bf16)),  # [E, n]
            "w_qkvT": wqT,
            "b_qkv": bq,
            "w_outT": woT,
            "b_out": bo,
        }
        for b in range(seq.shape[1])
    ]


def run(seq, W_qkv, b_qkv, W_out, b_out, trace=False):
    """Returns (out [n, bs, e] fp32, BassKernelResults)."""
    from concourse.bass_utils import run_bass_kernel_spmd

    seq = np.asarray(seq, np.float32)
    n, bs, e = seq.shape
    nc = _get_nc(n)
    res = run_bass_kernel_spmd(
        nc,
        _in_maps(seq, W_qkv, b_qkv, W_out, b_out),
        core_ids=list(range(N_CORES)),
        trace=trace,
    )
    out = np.empty((n, bs, e), np.float32)
    for b in range(bs):
        out[:, b, :] = res.results[b]["y"]
    return out, res


def kernel(seq, W_qkv, b_qkv, W_out, b_out):
    out, _ = run(seq, W_qkv, b_qkv, W_out, b_out)
    return out


# revision 14
# speedup vs baseline: 1.6630x; 1.0426x over previous
"""Multi-head self-attention Trainium2 kernel (Bass/Tile), batch-sharded SPMD.

Problem: seq [2048, 8, 512] fp32, fused QKV (W_qkv [1536,512], b_qkv [1536]),
H=8 heads of HD=64, full softmax attention, out proj (W_out [512,512], b_out).

Sharding: batch (bs=8) across 8 NeuronCores, one batch element per core.
No collectives; host scatters seq[:, b, :] (pre-transposed to [e, n]) and
gathers y -> [n, bs, e]. Weights are pre-transposed on host too, so no
on-chip transposes at all.

Per-core dataflow (n=2048, E=512):
  xT    [e, n]   <- DMA fp32, cast bf16 on DVE
  qkT   [f, n]   <- WqkvT.T @ xT  (f in [0,1024): q|k features; head-pairs
                    per 128-row tile: rows 0:64 head 2p, 64:128 head 2p+1)
  v     [n, f]   <- xT.T @ WvT (+bias via ones-lhsT matmul)
  per head pair p, per q-chunk (512 cols), k-blocks in batches of KBATCH:
    scoresT[k,q]: row-tiled PAIR matmuls (K=64 halves run concurrently)
    exp on ScalarE (scale=1/8, no max subtraction: |s| < ~4, exp safe)
    outT[hd,q] += v[k,hd].T @ exp   (col-tiled pair: M=64 at cols 0/64)
    denom      += ones[k,64].T @ exp (PE broadcasts denom over 64 rows)
    outT_norm   = outT * reciprocal(denom)  (both PSUM tiles double-buffered
                  so the slow DVE reciprocal stays off the PE critical path)
  y[n, f] = outT.T @ WoutT + b_out (ones-lhsT matmul)
"""

import numpy as np

import concourse.bass as bass
import concourse.mybir as mybir
import concourse.tile as tile
from concourse import bacc

F32 = mybir.dt.float32
BF16 = mybir.dt.bfloat16

N_SEQ, BS, E, H, HD = 2048, 8, 512, 8, 64
N_CORES = 8
KBATCH = 2  # k-blocks per scores PSUM tile (2+2 banks + 2*out + 2*denom = 8)


def _emit(tc, nc, xT_d, w_qkvT, b_qkv, w_outT, b_out, y, n):
    NB = n // 128   # token blocks
    QC = n // 512   # q chunks
    KB = n // 128   # k blocks
    EC = E // 128   # e chunks

    persist_cm = tc.tile_pool(name="persist", bufs=1)
    persist = persist_cm.__enter__()

    ones_col = persist.tile([128, 64], BF16, tag="ones_col", name="ones_col")
    nc.vector.memset(ones_col, 1.0)
    ones_row = persist.tile([1, 128], BF16, tag="ones_row", name="ones_row")
    nc.vector.memset(ones_row, 1.0)

    # biases: b_qkv[0:1024] per-partition layout [128, fb]; v/out biases as rows
    bqk = persist.tile([128, 8], F32, tag="bqk", name="bqk")
    nc.sync.dma_start(out=bqk, in_=b_qkv[0:1024].rearrange("(a b) -> b a", b=128))
    bv_f = persist.tile([1, 512], F32, tag="bv_f", name="bv_f")
    nc.sync.dma_start(out=bv_f, in_=b_qkv[1024:1536].unsqueeze(0))
    bv = persist.tile([1, 512], BF16, tag="bv", name="bv")
    nc.vector.tensor_copy(bv, bv_f)
    bo_f = persist.tile([1, 512], F32, tag="bo_f", name="bo_f")
    nc.sync.dma_start(out=bo_f, in_=b_out.unsqueeze(0))
    bo = persist.tile([1, 512], BF16, tag="bo", name="bo")
    nc.vector.tensor_copy(bo, bo_f)

    # persistent bf16 operands
    xT = persist.tile([128, EC, n], BF16, tag="xT", name="xT")
    wqkvT = persist.tile([128, EC, 1536], BF16, tag="wqkvT", name="wqkvT")
    woutT = persist.tile([128, EC, 512], BF16, tag="woutT", name="woutT")
    qkT = [persist.tile([128, n], BF16, tag=f"qkT{i}", name=f"qkT{i}") for i in range(8)]
    v_sb = [persist.tile([128, 512], BF16, tag=f"v{i}", name=f"v{i}") for i in range(NB)]
    outT = [persist.tile([128, n], BF16, tag=f"outT{p}", name=f"outT{p}") for p in range(4)]

    # ---------------- phase 0: load (bf16, pre-transposed on host) + QKV ----
    with (
        tc.tile_pool(name="pqkv", bufs=4, space="PSUM") as pqkv_pool,
    ):
        for j in range(EC):
            nc.sync.dma_start(
                out=wqkvT[:, j, :], in_=w_qkvT[j * 128:(j + 1) * 128, :]
            )
            nc.sync.dma_start(out=xT[:, j, :], in_=xT_d[j * 128:(j + 1) * 128, :])
        for j in range(EC):
            nc.sync.dma_start(
                out=woutT[:, j, :], in_=w_outT[j * 128:(j + 1) * 128, :]
            )

        def emit_qk(fb):
            for ncol in range(QC):
                pq = pqkv_pool.tile([128, 512], F32, tag="qk", name="pq")
                for j in range(EC):
                    nc.tensor.matmul(
                        pq,
                        lhsT=wqkvT[:, j, fb * 128:(fb + 1) * 128],
                        rhs=xT[:, j, ncol * 512:(ncol + 1) * 512],
                        start=(j == 0),
                        stop=(j == EC - 1),
                    )
                nc.vector.tensor_scalar_add(
                    qkT[fb][:, ncol * 512:(ncol + 1) * 512], pq, bqk[:, fb:fb + 1]
                )

        def emit_v(nb):
            pv = pqkv_pool.tile([128, 512], F32, tag="v", name="pv")
            for j in range(EC):
                nc.tensor.matmul(
                    pv,
                    lhsT=xT[:, j, nb * 128:(nb + 1) * 128],
                    rhs=wqkvT[:, j, 1024:1536],
                    start=(j == 0),
                    stop=False,
                )
            nc.tensor.matmul(pv, lhsT=ones_row, rhs=bv, start=False, stop=True)
            nc.vector.tensor_copy(v_sb[nb], pv)

        emit_qk(0)
        emit_qk(4)
        for nb in range(NB):
            emit_v(nb)
        for fb in (1, 5, 2, 6, 3, 7):
            emit_qk(fb)

    # ---------------- phase 1: attention ----------------
    # 3-kb cycles over a single 6-bank scores tensor: kb0/kb1 (A,B interleaved)
    # in banks 0-3 -> one 2048-elem exp; kb2 in banks 4-5 -> one 1024-elem exp.
    # The second exp hides the PE time of av(cycle)+scores(next cycle), so
    # ScalarE stays saturated. o/d single-banked; reciprocal_approx_fast makes
    # the qc-boundary normalize cheap enough to hide behind next-qc scores.
    cycles = [(0,)] + [tuple(range(s, s + 3)) for s in range(1, KB, 3)]
    with (
        tc.tile_pool(name="ps", bufs=1, space="PSUM") as s_pool,
        tc.tile_pool(name="po", bufs=1, space="PSUM") as o_pool,
        tc.tile_pool(name="se", bufs=3) as e_pool,
        tc.tile_pool(name="sr", bufs=2) as r_pool,
        tc.tile_pool(name="sy", bufs=4) as y_pool,
    ):
        def emit_final(nb, ftag):
            pf = o_pool.tile([128, 512], F32, tag=ftag, name="pf")
            for pp in range(4):
                nc.tensor.matmul(
                    pf, lhsT=outT[pp][:, nb * 128:(nb + 1) * 128],
                    rhs=woutT[:, pp, :], start=(pp == 0), stop=False,
                )
            nc.tensor.matmul(pf, lhsT=ones_row, rhs=bo, start=False, stop=True)
            ys = y_pool.tile([128, 512], F32, tag="y", name="ys")
            nc.vector.tensor_copy(ys, pf)
            nc.sync.dma_start(out=y[nb * 128:(nb + 1) * 128, :], in_=ys)

        for p in range(4):
            qa = qkT[p]
            ka = qkT[4 + p]
            for qc in range(QC):
                qs = slice(qc * 512, (qc + 1) * 512)
                po = o_pool.tile([128, 512], F32, tag="o", name="po")
                pd = o_pool.tile([128, 512], F32, tag="d", name="pd")

                def scores(S, slot, kb):
                    ks = slice(kb * 128, (kb + 1) * 128)
                    nc.tensor.matmul(
                        S[:, 2 * slot, :], lhsT=ka[0:64, ks], rhs=qa[0:64, qs],
                        start=True, stop=True,
                    )
                    nc.tensor.matmul(
                        S[:, 2 * slot + 1, :], lhsT=ka[64:128, ks], rhs=qa[64:128, qs],
                        start=True, stop=True,
                    )

                def av(e, slot, kb):
                    first, last = (kb == 0), (kb == KB - 1)
                    eA = e[:, 2 * slot, :]
                    eB = e[:, 2 * slot + 1, :]
                    nc.tensor.matmul(
                        po[0:64, :], lhsT=v_sb[kb][:, p * 128:p * 128 + 64],
                        rhs=eA, start=first, stop=last, skip_group_check=True,
                    )
                    nc.tensor.matmul(
                        po[64:128, :], lhsT=v_sb[kb][:, p * 128 + 64:(p + 1) * 128],
                        rhs=eB, start=first, stop=last, skip_group_check=True,
                    )
                    nc.tensor.matmul(
                        pd[0:64, :], lhsT=ones_col, rhs=eA,
                        start=first, stop=last, skip_group_check=True,
                    )
                    nc.tensor.matmul(
                        pd[64:128, :], lhsT=ones_col, rhs=eB,
                        start=first, stop=last, skip_group_check=True,
                    )

                pend = []  # av work deferred by one cycle to keep ScalarE fed
                for cyc in cycles:
                    if len(cyc) == 3:
                        k0, k1, k2 = cyc
                        S01 = s_pool.tile([128, 4, 512], F32, tag="s01", name="S01")
                        scores(S01, 0, k0)
                        scores(S01, 1, k1)
                        e1 = e_pool.tile([128, 4, 512], BF16, tag="e4", name="e1")
                        nc.scalar.activation(
                            e1, S01, mybir.ActivationFunctionType.Exp, scale=0.125,
                        )
                        S2 = s_pool.tile([128, 2, 512], F32, tag="s2", name="S2")
                        scores(S2, 0, k2)
                        e2 = e_pool.tile([128, 2, 512], BF16, tag="e2", name="e2")
                        nc.scalar.activation(
                            e2, S2, mybir.ActivationFunctionType.Exp, scale=0.125,
                        )
                        for args in pend:
                            av(*args)
                        pend = [(e1, 0, k0), (e1, 1, k1), (e2, 0, k2)]
                    else:
                        (k0,) = cyc
                        S2 = s_pool.tile([128, 2, 512], F32, tag="s2", name="S2")
                        scores(S2, 0, k0)
                        e2 = e_pool.tile([128, 2, 512], BF16, tag="e2", name="e2")
                        nc.scalar.activation(
                            e2, S2, mybir.ActivationFunctionType.Exp, scale=0.125,
                        )
                        for args in pend:
                            av(*args)
                        pend = [(e2, 0, k0)]
                for args in pend:
                    av(*args)
                rc = r_pool.tile([128, 512], F32, tag="rc", name="rc")
                nc.vector.reciprocal_approx_fast(rc, pd)
                nc.vector.tensor_mul(outT[p][:, qs], po, rc)
                if p == 3:
                    for i, nb in enumerate(range(qc * 4, qc * 4 + 4)):
                        emit_final(nb, "o" if i % 2 == 0 else "d")

    persist_cm.__exit__(None, None, None)


def build(n=N_SEQ):
    nc = bacc.Bacc("TRN2", target_bir_lowering=False, debug=False)
    xT_d = nc.dram_tensor("xT", [E, n], BF16, kind="ExternalInput").ap()
    w_qkvT = nc.dram_tensor("w_qkvT", [E, 3 * E], BF16, kind="ExternalInput").ap()
    b_qkv = nc.dram_tensor("b_qkv", [3 * E], F32, kind="ExternalInput").ap()
    w_outT = nc.dram_tensor("w_outT", [E, E], BF16, kind="ExternalInput").ap()
    b_out = nc.dram_tensor("b_out", [E], F32, kind="ExternalInput").ap()
    y = nc.dram_tensor("y", [n, E], F32, kind="ExternalOutput").ap()
    with tile.TileContext(nc) as tc:
        _emit(tc, nc, xT_d, w_qkvT, b_qkv, w_outT, b_out, y, n)
    nc.compile()
    return nc


_NC_CACHE = {}


def _get_nc(n):
    if n not in _NC_CACHE:
        _NC_CACHE[n] = build(n)
    return _NC_CACHE[n]


def _in_maps(seq, W_qkv, b_qkv, W_out, b_out):
    import ml_dtypes

    bf16 = ml_dtypes.bfloat16
    seq = np.asarray(seq, np.float32)
    wqT = np.ascontiguousarray(np.asarray(W_qkv, np.float32).T.astype(bf16))
    bq = np.ascontiguousarray(np.asarray(b_qkv, np.float32))
    woT = np.ascontiguousarray(np.asarray(W_out, np.float32).T.astype(bf16))
    bo = np.ascontiguousarray(np.asarray(b_out, np.float32))
    return [
        {
            "xT": np.ascontiguousarray(seq[:, b, :].T.astype(bf16)),  # [E, n]
            "w_qkvT": wqT,
            "b_qkv": bq,
            "w_outT": woT,
            "b_out": bo,
        }
        for b in range(seq.shape[1])
    ]


def run(seq, W_qkv, b_qkv, W_out, b_out, trace=False):
    """Returns (out [n, bs, e] fp32, BassKernelResults)."""
    from concourse.bass_utils import run_bass_kernel_spmd

    seq = np.asarray(seq, np.float32)
    n, bs, e = seq.shape
    nc = _get_nc(n)
    res = run_bass_kernel_spmd(
        nc,
        _in_maps(seq, W_qkv, b_qkv, W_out, b_out),
        core_ids=list(range(N_CORES)),
        trace=trace,
    )
    out = np.empty((n, bs, e), np.float32)
    for b in range(bs):
        out[:, b, :] = res.results[b]["y"]
    return out, res


def kernel(seq, W_qkv, b_qkv, W_out, b_out):
    out, _ = run(seq, W_qkv, b_qkv, W_out, b_out)
    return out


# revision 17
# speedup vs baseline: 1.6789x; 1.0096x over previous
"""Multi-head self-attention Trainium2 kernel (Bass/Tile), batch-sharded SPMD.

Problem: seq [2048, 8, 512] fp32, fused QKV (W_qkv [1536,512], b_qkv [1536]),
H=8 heads of HD=64, full softmax attention, out proj (W_out [512,512], b_out).

Sharding: batch (bs=8) across 8 NeuronCores, one batch element per core.
No collectives; host scatters seq[:, b, :] (pre-transposed to [e, n]) and
gathers y -> [n, bs, e]. Weights are pre-transposed on host too, so no
on-chip transposes at all.

Per-core dataflow (n=2048, E=512):
  xT    [e, n]   <- DMA fp32, cast bf16 on DVE
  qkT   [f, n]   <- WqkvT.T @ xT  (f in [0,1024): q|k features; head-pairs
                    per 128-row tile: rows 0:64 head 2p, 64:128 head 2p+1)
  v     [n, f]   <- xT.T @ WvT (+bias via ones-lhsT matmul)
  per head pair p, per q-chunk (512 cols), k-blocks in batches of KBATCH:
    scoresT[k,q]: row-tiled PAIR matmuls (K=64 halves run concurrently)
    exp on ScalarE (scale=1/8, no max subtraction: |s| < ~4, exp safe)
    outT[hd,q] += v[k,hd].T @ exp   (col-tiled pair: M=64 at cols 0/64)
    denom      += ones[k,64].T @ exp (PE broadcasts denom over 64 rows)
    outT_norm   = outT * reciprocal(denom)  (both PSUM tiles double-buffered
                  so the slow DVE reciprocal stays off the PE critical path)
  y[n, f] = outT.T @ WoutT + b_out (ones-lhsT matmul)
"""

import numpy as np

import concourse.bass as bass
import concourse.mybir as mybir
import concourse.tile as tile
from concourse import bacc

F32 = mybir.dt.float32
BF16 = mybir.dt.bfloat16

N_SEQ, BS, E, H, HD = 2048, 8, 512, 8, 64
N_CORES = 8
KBATCH = 2  # k-blocks per scores PSUM tile (2+2 banks + 2*out + 2*denom = 8)


def _emit(tc, nc, xT_d, w_qkvT, b_qkv, w_outT, b_out, y, n):
    NB = n // 128   # token blocks
    QC = n // 512   # q chunks
    KB = n // 128   # k blocks
    EC = E // 128   # e chunks

    persist_cm = tc.tile_pool(name="persist", bufs=1)
    persist = persist_cm.__enter__()

    ones_col = persist.tile([128, 64], BF16, tag="ones_col", name="ones_col")
    nc.vector.memset(ones_col, 1.0)
    ones_row = persist.tile([1, 128], BF16, tag="ones_row", name="ones_row")
    nc.vector.memset(ones_row, 1.0)

    # biases: b_qkv[0:1024] per-partition layout [128, fb]; v/out biases as rows
    bqk = persist.tile([128, 8], F32, tag="bqk", name="bqk")
    nc.sync.dma_start(out=bqk, in_=b_qkv[0:1024].rearrange("(a b) -> b a", b=128))
    bv_f = persist.tile([1, 512], F32, tag="bv_f", name="bv_f")
    nc.sync.dma_start(out=bv_f, in_=b_qkv[1024:1536].unsqueeze(0))
    bv = persist.tile([1, 512], BF16, tag="bv", name="bv")
    nc.vector.tensor_copy(bv, bv_f)
    bo_f = persist.tile([1, 512], F32, tag="bo_f", name="bo_f")
    nc.sync.dma_start(out=bo_f, in_=b_out.unsqueeze(0))
    bo = persist.tile([1, 512], BF16, tag="bo", name="bo")
    nc.vector.tensor_copy(bo, bo_f)

    # persistent bf16 operands
    xT = persist.tile([128, EC, n], BF16, tag="xT", name="xT")
    wqkvT = persist.tile([128, EC, 1536], BF16, tag="wqkvT", name="wqkvT")
    woutT = persist.tile([128, EC, 512], BF16, tag="woutT", name="woutT")
    qkT = [persist.tile([128, n], BF16, tag=f"qkT{i}", name=f"qkT{i}") for i in range(8)]
    v_sb = [persist.tile([128, 512], BF16, tag=f"v{i}", name=f"v{i}") for i in range(NB)]
    outT = [persist.tile([128, n], BF16, tag=f"outT{p}", name=f"outT{p}") for p in range(4)]

    # ---------------- phase 0: load (bf16, pre-transposed on host) + QKV ----
    with (
        tc.tile_pool(name="pqkv", bufs=4, space="PSUM") as pqkv_pool,
    ):
        for j in range(EC):
            nc.sync.dma_start(
                out=wqkvT[:, j, :], in_=w_qkvT[j * 128:(j + 1) * 128, :]
            )
            nc.scalar.dma_start(out=xT[:, j, :], in_=xT_d[j * 128:(j + 1) * 128, :])
        for j in range(EC):
            nc.sync.dma_start(
                out=woutT[:, j, :], in_=w_outT[j * 128:(j + 1) * 128, :]
            )

        def emit_qk(fb):
            for ncol in range(QC):
                pq = pqkv_pool.tile([128, 512], F32, tag="qk", name="pq")
                for j in range(EC):
                    nc.tensor.matmul(
                        pq,
                        lhsT=wqkvT[:, j, fb * 128:(fb + 1) * 128],
                        rhs=xT[:, j, ncol * 512:(ncol + 1) * 512],
                        start=(j == 0),
                        stop=(j == EC - 1),
                    )
                nc.vector.tensor_scalar_add(
                    qkT[fb][:, ncol * 512:(ncol + 1) * 512], pq, bqk[:, fb:fb + 1]
                )

        def emit_v(nb):
            pv = pqkv_pool.tile([128, 512], F32, tag="v", name="pv")
            for j in range(EC):
                nc.tensor.matmul(
                    pv,
                    lhsT=xT[:, j, nb * 128:(nb + 1) * 128],
                    rhs=wqkvT[:, j, 1024:1536],
                    start=(j == 0),
                    stop=False,
                )
            nc.tensor.matmul(pv, lhsT=ones_row, rhs=bv, start=False, stop=True)
            nc.vector.tensor_copy(v_sb[nb], pv)

        emit_qk(0)
        emit_qk(4)
        for nb in range(NB):
            emit_v(nb)
        for fb in (1, 5, 2, 6, 3, 7):
            emit_qk(fb)

    # ---------------- phase 1: attention ----------------
    # 3-kb cycles over a single 6-bank scores tensor: kb0/kb1 (A,B interleaved)
    # in banks 0-3 -> one 2048-elem exp; kb2 in banks 4-5 -> one 1024-elem exp.
    # The second exp hides the PE time of av(cycle)+scores(next cycle), so
    # ScalarE stays saturated. o/d single-banked; reciprocal_approx_fast makes
    # the qc-boundary normalize cheap enough to hide behind next-qc scores.
    cycles = [(0,)] + [tuple(range(s, s + 3)) for s in range(1, KB, 3)]
    with (
        tc.tile_pool(name="ps", bufs=1, space="PSUM") as s_pool,
        tc.tile_pool(name="po", bufs=1, space="PSUM") as o_pool,
        tc.tile_pool(name="se", bufs=3) as e_pool,
        tc.tile_pool(name="sr", bufs=2) as r_pool,
        tc.tile_pool(name="sy", bufs=4) as y_pool,
    ):
        def emit_final(nb, ftag):
            pf = o_pool.tile([128, 512], F32, tag=ftag, name="pf")
            for pp in range(4):
                nc.tensor.matmul(
                    pf, lhsT=outT[pp][:, nb * 128:(nb + 1) * 128],
                    rhs=woutT[:, pp, :], start=(pp == 0), stop=False,
                )
            nc.tensor.matmul(pf, lhsT=ones_row, rhs=bo, start=False, stop=True)
            ys = y_pool.tile([128, 512], F32, tag="y", name="ys")
            nc.vector.tensor_copy(ys, pf)
            nc.sync.dma_start(out=y[nb * 128:(nb + 1) * 128, :], in_=ys)

        for p in range(4):
            qa = qkT[p]
            ka = qkT[4 + p]
            work = []  # closures deferred until after the next cycle's exps

            def flush(cap=6):
                m = min(cap, len(work))
                for w in work[:m]:
                    w()
                del work[:m]

            for qc in range(QC):
                qs = slice(qc * 512, (qc + 1) * 512)
                po = o_pool.tile([128, 512], F32, tag="o", name="po")
                pd = o_pool.tile([128, 512], F32, tag="d", name="pd")

                def scores(S, slot, kb):
                    ks = slice(kb * 128, (kb + 1) * 128)
                    nc.tensor.matmul(
                        S[:, 2 * slot, :], lhsT=ka[0:64, ks], rhs=qa[0:64, qs],
                        start=True, stop=True,
                    )
                    nc.tensor.matmul(
                        S[:, 2 * slot + 1, :], lhsT=ka[64:128, ks], rhs=qa[64:128, qs],
                        start=True, stop=True,
                    )

                def av(e, slot, kb, po=po, pd=pd, p=p):
                    first, last = (kb == 0), (kb == KB - 1)
                    eA = e[:, 2 * slot, :]
                    eB = e[:, 2 * slot + 1, :]
                    nc.tensor.matmul(
                        po[0:64, :], lhsT=v_sb[kb][:, p * 128:p * 128 + 64],
                        rhs=eA, start=first, stop=last, skip_group_check=True,
                    )
                    nc.tensor.matmul(
                        po[64:128, :], lhsT=v_sb[kb][:, p * 128 + 64:(p + 1) * 128],
                        rhs=eB, start=first, stop=last, skip_group_check=True,
                    )
                    nc.tensor.matmul(
                        pd[0:64, :], lhsT=ones_col, rhs=eA,
                        start=first, stop=last, skip_group_check=True,
                    )
                    nc.tensor.matmul(
                        pd[64:128, :], lhsT=ones_col, rhs=eB,
                        start=first, stop=last, skip_group_check=True,
                    )

                def normalize(po=po, pd=pd, p=p, qs=qs):
                    rc = r_pool.tile([128, 512], F32, tag="rc", name="rc")
                    nc.vector.reciprocal_approx_fast(rc, pd)
                    nc.vector.tensor_mul(outT[p][:, qs], po, rc)

                for cyc in cycles:
                    if len(cyc) == 3:
                        k0, k1, k2 = cyc
                        S01 = s_pool.tile([128, 4, 512], F32, tag="s01", name="S01")
                        scores(S01, 0, k0)
                        scores(S01, 1, k1)
                        e1 = e_pool.tile([128, 4, 512], BF16, tag="e4", name="e1")
                        nc.scalar.activation(
                            e1, S01, mybir.ActivationFunctionType.Exp, scale=0.125,
                        )
                        S2 = s_pool.tile([128, 2, 512], F32, tag="s2", name="S2")
                        scores(S2, 0, k2)
                        e2 = e_pool.tile([128, 2, 512], BF16, tag="e2", name="e2")
                        nc.scalar.activation(
                            e2, S2, mybir.ActivationFunctionType.Exp, scale=0.125,
                        )
                        flush()
                        work.extend([
                            lambda e1=e1, k0=k0, av=av: av(e1, 0, k0),
                            lambda e1=e1, k1=k1, av=av: av(e1, 1, k1),
                            lambda e2=e2, k2=k2, av=av: av(e2, 0, k2),
                        ])
                    else:
                        (k0,) = cyc
                        S2 = s_pool.tile([128, 2, 512], F32, tag="s2", name="S2")
                        scores(S2, 0, k0)
                        e2 = e_pool.tile([128, 2, 512], BF16, tag="e2", name="e2")
                        nc.scalar.activation(
                            e2, S2, mybir.ActivationFunctionType.Exp, scale=0.125,
                        )
                        flush()
                        work.extend([lambda e2=e2, k0=k0, av=av: av(e2, 0, k0)])
                # normalization (and, on the last pair, the output projection
                # rows that just became complete) joins the deferred queue so
                # the next qc's scores/exps stay ahead of it
                work.append(normalize)
                if p == 3:
                    for i, nb in enumerate(range(qc * 4, qc * 4 + 4)):
                        work.append(
                            lambda nb=nb, t=("o" if i % 2 == 0 else "d"):
                                emit_final(nb, t)
                        )
            while work:
                flush()
    persist_cm.__exit__(None, None, None)


def build(n=N_SEQ):
    nc = bacc.Bacc("TRN2", target_bir_lowering=False, debug=False)
    xT_d = nc.dram_tensor("xT", [E, n], BF16, kind="ExternalInput").ap()
    w_qkvT = nc.dram_tensor("w_qkvT", [E, 3 * E], BF16, kind="ExternalInput").ap()
    b_qkv = nc.dram_tensor("b_qkv", [3 * E], F32, kind="ExternalInput").ap()
    w_outT = nc.dram_tensor("w_outT", [E, E], BF16, kind="ExternalInput").ap()
    b_out = nc.dram_tensor("b_out", [E], F32, kind="ExternalInput").ap()
    y = nc.dram_tensor("y", [n, E], F32, kind="ExternalOutput").ap()
    with tile.TileContext(nc) as tc:
        _emit(tc, nc, xT_d, w_qkvT, b_qkv, w_outT, b_out, y, n)
    nc.compile()
    return nc


_NC_CACHE = {}


def _get_nc(n):
    if n not in _NC_CACHE:
        _NC_CACHE[n] = build(n)
    return _NC_CACHE[n]


def _in_maps(seq, W_qkv, b_qkv, W_out, b_out):
    import ml_dtypes

    bf16 = ml_dtypes.bfloat16
    seq = np.asarray(seq, np.float32)
    wqT = np.ascontiguousarray(np.asarray(W_qkv, np.float32).T.astype(bf16))
    bq = np.ascontiguousarray(np.asarray(b_qkv, np.float32))
    woT = np.ascontiguousarray(np.asarray(W_out, np.float32).T.astype(bf16))
    bo = np.ascontiguousarray(np.asarray(b_out, np.float32))
    return [
        {
            "xT": np.ascontiguousarray(seq[:, b, :].T.astype(bf16)),  # [E, n]
            "w_qkvT": wqT,
            "b_qkv": bq,
            "w_outT": woT,
            "b_out": bo,
        }
        for b in range(seq.shape[1])
    ]


def run(seq, W_qkv, b_qkv, W_out, b_out, trace=False):
    """Returns (out [n, bs, e] fp32, BassKernelResults)."""
    from concourse.bass_utils import run_bass_kernel_spmd

    seq = np.asarray(seq, np.float32)
    n, bs, e = seq.shape
    nc = _get_nc(n)
    res = run_bass_kernel_spmd(
        nc,
        _in_maps(seq, W_qkv, b_qkv, W_out, b_out),
        core_ids=list(range(N_CORES)),
        trace=trace,
    )
    out = np.empty((n, bs, e), np.float32)
    for b in range(bs):
        out[:, b, :] = res.results[b]["y"]
    return out, res


def kernel(seq, W_qkv, b_qkv, W_out, b_out):
    out, _ = run(seq, W_qkv, b_qkv, W_out, b_out)
    return out
